# revision 14
# baseline (speedup 1.0000x reference)
"""Trainium2 Bass kernel for nn_MultiHeadAttention (B=2,S=2048,D=1024,H=16, RoPE+ALiBi+causal).

Strategy: head-parallel sharding across 8 NeuronCores (2 heads/core, both batches).
Host folds mask+bias into a per-head multiplicative exp(bias) tensor, pre-transposes
x and the bias, and sums the per-core partial output projections.
"""
import sys, os

for _p in ("/root/.axon_site/_ro/trn_rl_repo", "/opt/trn_rl_repo"):
    if os.path.isdir(_p) and _p not in sys.path:
        sys.path.insert(0, _p)

import numpy as np
import concourse.bass as bass
import concourse.mybir as mybir
import concourse.tile as tile
from concourse import bacc
from concourse.bass_utils import run_bass_kernel_spmd

F32 = mybir.dt.float32
F32R = mybir.dt.float32r

B, S, D, H = 2, 2048, 1024, 16
DK = D // H            # 64
NCORES = 8
HPC = H // NCORES      # 2 heads per core
FD = HPC * DK          # 128 ctx features per core
R = B * S              # 4096 token rows
RT = R // 128          # 32 r-tiles
QB = 512               # q-block size
NQB = S // QB          # 4 q-blocks per batch


def _build(causal: bool, qk_bias: bool, stage: int = 2):
    """Build + compile the per-core SPMD kernel. Returns the compiled Bacc."""
    nc = bacc.Bacc()

    xT = nc.dram_tensor("xT", (D, R), F32R, kind="ExternalInput")
    wcat = nc.dram_tensor("wcat", (D, 3 * FD), F32R, kind="ExternalInput")
    woB = nc.dram_tensor("woB", (2 * DK, D), F32R, kind="ExternalInput")
    expb = nc.dram_tensor("expb", (HPC, S, S), F32, kind="ExternalInput")
    cosp = nc.dram_tensor("cosp", (128, RT * (DK // 2)), F32, kind="ExternalInput")
    sinp = nc.dram_tensor("sinp", (128, RT * (DK // 2)), F32, kind="ExternalInput")
    ident = nc.dram_tensor("ident", (128, 128), F32R, kind="ExternalInput")
    onesd = nc.dram_tensor("onesd", (128, 64), F32R, kind="ExternalInput")
    if qk_bias:
        bropeq = nc.dram_tensor("bropeq", (128, R), F32, kind="ExternalInput")
        bropek = nc.dram_tensor("bropek", (128, R), F32, kind="ExternalInput")
    out = nc.dram_tensor("out", (R, D), F32, kind="ExternalOutput")

    with tile.TileContext(nc) as tc:
        import contextlib
        ctx = contextlib.ExitStack()
        with ctx:
            consts = ctx.enter_context(tc.tile_pool(name="consts", bufs=1))
            persist = ctx.enter_context(tc.tile_pool(name="persist", bufs=1))

            # --- constants ---
            id_sb = consts.tile([128, 128], F32R)
            nc.sync.dma_start(out=id_sb, in_=ident[:, :])
            wc_sb = [consts.tile([128, 3 * FD], F32R, tag=f"wc{ct}", name=f"wc{ct}") for ct in range(8)]
            for ct in range(8):
                nc.sync.dma_start(out=wc_sb[ct], in_=wcat[ct * 128:(ct + 1) * 128, :])
            wo_sb = consts.tile([2 * DK, D], F32R, tag="wo", name="wo")
            nc.sync.dma_start(out=wo_sb, in_=woB[:, :])
            cos_sb = consts.tile([128, RT * 32], F32)
            sin_sb = consts.tile([128, RT * 32], F32)
            nc.sync.dma_start(out=cos_sb, in_=cosp[:, :])
            nc.sync.dma_start(out=sin_sb, in_=sinp[:, :])
            ones_sb = consts.tile([128, DK], F32R)
            nc.sync.dma_start(out=ones_sb, in_=onesd[:, :])

            # --- persistent activation tensors ---
            QT = persist.tile([128, R], F32R, tag="QT")   # rows: h*64 + d, cols: b*2048+s
            KT = persist.tile([128, R], F32R, tag="KT")
            vaug = {}
            for rt in range(RT):
                for hh in range(HPC):
                    vaug[(rt, hh)] = persist.tile([128, DK + 1], F32R, tag=f"va{rt}_{hh}", name=f"va{rt}_{hh}")

            # =================== Phase 1: QKV projection + RoPE + transposes ===================
            with tc.tile_pool(name="p1x", bufs=3) as p1x, \
                 tc.tile_pool(name="p1n", bufs=1) as p1n, \
                 tc.tile_pool(name="p1s", bufs=4) as p1s, \
                 tc.tile_pool(name="p1ps", bufs=3, space="PSUM") as p1ps, \
                 tc.tile_pool(name="p1pt", bufs=3, space="PSUM") as p1pt:

                qknat = p1n.tile([128, RT * 256], F32)  # col = rt*256 + {0:128 Q | 128:256 K}, free d-major
                qkrot = p1n.tile([128, RT * 256], F32R)

                GRP = 8  # r-tiles per rope group
                for g in range(RT // GRP):
                    for rt in range(g * GRP, (g + 1) * GRP):
                        xts = [p1x.tile([128, 128], F32R, tag=f"x{ct}", name=f"xt{rt}_{ct}") for ct in range(8)]
                        for ct in range(8):
                            nc.sync.dma_start(
                                out=xts[ct],
                                in_=xT[ct * 128:(ct + 1) * 128, rt * 128:(rt + 1) * 128])
                        pp = p1ps.tile([128, 3 * FD], F32, tag="prj")
                        for ct in range(8):
                            nc.tensor.matmul(pp, xts[ct], wc_sb[ct],
                                             start=(ct == 0), stop=(ct == 7))
                        # drain Q,K -> qknat (fp32); V -> vaug tiles (f32r)
                        nc.vector.tensor_copy(qknat[:, rt * 256: rt * 256 + 256], pp[:, 0:256])
                        for hh in range(HPC):
                            va = vaug[(rt, hh)]
                            nc.scalar.copy(va[:, 0:DK], pp[:, 2 * FD + hh * DK: 2 * FD + (hh + 1) * DK])
                            nc.sync.dma_start(out=va[:, DK:DK + 1], in_=onesd[:, 0:1])

                    # rope on group g: Q and K separately, 6 ops each
                    # AP dims over qknat: [part][rt(8) step 256][head(2) step 64][pair(32) step 2]
                    def sl(t, qk, eo, g=g):
                        a = t[:, :]
                        return bass.AP(
                            tensor=a.tensor,
                            offset=a.offset + (g * GRP * 256 + qk * 128 + eo),
                            ap=[a.ap[0], [256, GRP], [64, 2], [2, 32]],
                        )
                    def slc(t, g=g):  # cos/sin AP: [part][rt(8) step 32][head 0x2][pair step 1 x32]
                        a = t[:, :]
                        return bass.AP(
                            tensor=a.tensor,
                            offset=a.offset + g * GRP * 32,
                            ap=[a.ap[0], [32, GRP], [0, 2], [1, 32]],
                        )
                    for qk in range(2):
                        s1 = p1s.tile([128, GRP * 64], F32, tag="s1")
                        s2 = p1s.tile([128, GRP * 64], F32, tag="s2")
                        s3 = p1s.tile([128, GRP * 64], F32, tag="s3")
                        s4 = p1s.tile([128, GRP * 64], F32, tag="s4")
                        nc.vector.tensor_mul(s1, sl(qknat, qk, 0), slc(cos_sb))
                        nc.vector.tensor_mul(s2, sl(qknat, qk, 1), slc(sin_sb))
                        nc.vector.tensor_sub(sl(qkrot, qk, 0), s1, s2)
                        nc.vector.tensor_mul(s3, sl(qknat, qk, 0), slc(sin_sb))
                        nc.vector.tensor_mul(s4, sl(qknat, qk, 1), slc(cos_sb))
                        nc.vector.tensor_add(sl(qkrot, qk, 1), s3, s4)

                    # transposes for group g: (128r x 128d) -> (128d x 128r)
                    for rt in range(g * GRP, (g + 1) * GRP):
                        for qk, dst in ((0, QT), (1, KT)):
                            pt = p1pt.tile([128, 128], F32R, tag="pt")
                            nc.tensor.transpose(pt, qkrot[:, rt * 256 + qk * 128: rt * 256 + qk * 128 + 128], id_sb)
                            nc.scalar.copy(dst[:, rt * 128:(rt + 1) * 128], pt)

                if qk_bias:
                    brq = p1n.tile([128, R], F32, tag="brq")
                    brk = p1n.tile([128, R], F32, tag="brk")
                    nc.sync.dma_start(out=brq, in_=bropeq[:, :])
                    nc.sync.dma_start(out=brk, in_=bropek[:, :])
                    nc.vector.tensor_add(QT, QT, brq)
                    nc.vector.tensor_add(KT, KT, brk)

            if stage == 1:
                for i in range(4):
                    nc.sync.dma_start(out=out[i * 128:(i + 1) * 128, :], in_=QT[:, i * 1024:(i + 1) * 1024].bitcast(F32))
                    nc.sync.dma_start(out=out[512 + i * 128: 512 + (i + 1) * 128, :], in_=KT[:, i * 1024:(i + 1) * 1024].bitcast(F32))
                nc.sync.dma_start(out=out[1024:1152, 0:65], in_=vaug[(0, 0)].bitcast(F32))
                nc.sync.dma_start(out=out[1152:1280, 0:65], in_=vaug[(31, 1)].bitcast(F32))
            # =================== Phase 2: attention + output projection ===================
            if stage >= 2:
              with tc.tile_pool(name="p2eb", bufs=3) as p2eb, \
                 tc.tile_pool(name="p2e", bufs=3) as p2e, \
                 tc.tile_pool(name="p2a", bufs=3) as p2a, \
                 tc.tile_pool(name="p2c", bufs=2) as p2c, \
                 tc.tile_pool(name="p2o", bufs=3) as p2o, \
                 tc.tile_pool(name="psc", bufs=3, space="PSUM") as psc, \
                 tc.tile_pool(name="psx", bufs=1, space="PSUM") as psx, \
                 tc.tile_pool(name="pm", bufs=1, space="PSUM") as pm:

                for qb in range(NQB):
                    nkt = (qb + 1) * (QB // 128) if causal else S // 128
                    ctx_ps = {}
                    for hh in range(HPC):
                        for b in range(B):
                            ctx_ps[(hh, b)] = psx.tile([DK + 1, QB], F32, tag=f"ctx{hh}{b}", name=f"ctx{qb}_{hh}{b}")
                    for kt in range(nkt):
                        q_off = max(0, kt * 128 - qb * QB) if causal else 0
                        q_len = QB - q_off
                        for hh in range(HPC):
                            ebt = p2eb.tile([128, QB], F32, tag=f"eb{hh}")
                            nc.sync.dma_start(
                                out=ebt[:, 0:q_len],
                                in_=expb[hh, kt * 128:(kt + 1) * 128,
                                         qb * QB + q_off: (qb + 1) * QB])
                            for b in range(B):
                                scp = psc.tile([128, QB], F32, tag="sc")
                                nc.tensor.matmul(
                                    scp[:, 0:q_len],
                                    KT[hh * DK:(hh + 1) * DK, b * S + kt * 128: b * S + (kt + 1) * 128],
                                    QT[hh * DK:(hh + 1) * DK, b * S + qb * QB + q_off: b * S + (qb + 1) * QB],
                                    start=True, stop=True)
                                ex = p2e.tile([128, QB], F32, tag="ex")
                                nc.scalar.activation(ex[:, 0:q_len], scp[:, 0:q_len],
                                                     mybir.ActivationFunctionType.Exp)
                                at = p2a.tile([128, QB], F32R, tag="at")
                                nc.vector.tensor_mul(at[:, 0:q_len], ex[:, 0:q_len], ebt[:, 0:q_len])
                                nc.tensor.matmul(
                                    ctx_ps[(hh, b)][:, q_off:QB],
                                    vaug[(b * (S // 128) + kt, hh)],
                                    at[:, 0:q_len],
                                    start=(kt == 0), stop=(kt == nkt - 1))
                    if stage == 3:
                        for hh in range(HPC):
                            for b in range(B):
                                dt_sb = p2o.tile([DK + 1, QB], F32, tag="dmp", name=f"dmp{qb}{hh}{b}")
                                nc.vector.tensor_copy(dt_sb, ctx_ps[(hh, b)])
                                nc.sync.dma_start(
                                    out=out[(qb * 4 + hh * 2 + b) * 128: (qb * 4 + hh * 2 + b) * 128 + DK + 1, 0:QB],
                                    in_=dt_sb)
                        continue
                    # normalize + output projection
                    csc = {}
                    for b in range(B):
                        cboth = p2c.tile([2 * DK, QB], F32R, tag=f"cb{b}", name=f"cb{qb}{b}")
                        for hh in range(HPC):
                            cp = ctx_ps[(hh, b)]
                            rc = p2c.tile([DK + 1, QB], F32R, tag=f"rc{hh}{b}")
                            with nc.allow_low_precision(reason="f32r reciprocal for matmul rhs"):
                                nc.vector.reciprocal(rc[DK:DK + 1, :], cp[DK:DK + 1, :])
                            rbp = pm.tile([DK, QB], F32, tag="rb")
                            nc.tensor.matmul(rbp, ones_sb[DK:DK + 1, :], rc[DK:DK + 1, :],
                                             start=True, stop=True)
                            rb = p2c.tile([DK, QB], F32, tag=f"rb{hh}{b}")
                            nc.scalar.copy(rb, rbp)
                            if hh == 0:
                                nc.vector.tensor_mul(cboth[0:DK, :], cp[0:DK, :], rb)
                            else:
                                cs1 = p2c.tile([DK, QB], F32R, tag=f"cs{hh}{b}")
                                nc.vector.tensor_mul(cs1, cp[0:DK, :], rb)
                                nc.sync.dma_start(out=cboth[DK:2 * DK, :], in_=cs1)
                        csc[b] = cboth
                    if stage == 4:
                        for b in range(B):
                            for hh in range(HPC):
                                dt_sb = p2o.tile([DK, QB], F32, tag="dmp", name=f"dmp{qb}{hh}{b}")
                                nc.vector.tensor_copy(dt_sb, csc[b][hh * DK:(hh + 1) * DK, :].bitcast(F32))
                                nc.sync.dma_start(
                                    out=out[(qb * 4 + hh * 2 + b) * 128: (qb * 4 + hh * 2 + b) * 128 + DK, 0:QB],
                                    in_=dt_sb)
                        continue
                    for b in range(B):
                        for rs in range(QB // 128):
                            ot = p2o.tile([128, D], F32, tag="ot")
                            for eh in range(2):
                                op = psc.tile([128, 512], F32, tag="sc")
                                nc.tensor.matmul(op, csc[b][:, rs * 128:(rs + 1) * 128],
                                                 wo_sb[:, eh * 512:(eh + 1) * 512],
                                                 start=True, stop=True)
                                if eh == 0:
                                    nc.vector.tensor_copy(ot[:, 0:512], op)
                                else:
                                    nc.scalar.copy(ot[:, 512:1024], op)
                            nc.sync.dma_start(
                                out=out[b * S + qb * QB + rs * 128: b * S + qb * QB + (rs + 1) * 128, :],
                                in_=ot)
    nc.compile()
    return nc


_CACHE = {}


def _get_kernel(causal: bool, qk_bias: bool):
    key = (causal, qk_bias)
    if key not in _CACHE:
        _CACHE[key] = _build(causal, qk_bias)
    return _CACHE[key]


def _host_prep(x, mask, bias, rope_freqs, Wq, bq, Wk, bk, Wv, bv, Wo, bo, causal):
    """Build the 8 per-core input maps."""
    xf = np.ascontiguousarray(x.reshape(R, D).T.astype(np.float32))  # (D, R)
    cosf = np.cos(rope_freqs.astype(np.float32))  # (S, 32)
    sinf = np.sin(rope_freqs.astype(np.float32))
    # packed cos/sin: (128, RT*32): [p, rt*32+i] = cos(freqs[(rt*128+p) % S, i])
    rr = np.arange(R)
    cs_full = cosf[rr % S]  # (R, 32)
    sn_full = sinf[rr % S]
    cosp = np.ascontiguousarray(
        cs_full.reshape(RT, 128, 32).transpose(1, 0, 2).reshape(128, RT * 32))
    sinp = np.ascontiguousarray(
        sn_full.reshape(RT, 128, 32).transpose(1, 0, 2).reshape(128, RT * 32))
    identm = np.eye(128, dtype=np.float32)

    qk_bias = bool(np.any(bq) or np.any(bk))
    maskT = (mask != 0).T  # (k, q)

    in_maps = []
    for c in range(NCORES):
        h0 = c * HPC
        fsl = slice(c * FD, (c + 1) * FD)
        wq = Wq[fsl, :].astype(np.float32) / np.sqrt(np.float32(DK))
        wk = Wk[fsl, :].astype(np.float32)
        wv = Wv[fsl, :].astype(np.float32)
        wcat = np.ascontiguousarray(np.concatenate([wq, wk, wv], axis=0).T)  # (D, 384)
        wob = np.ascontiguousarray(Wo[:, c * FD: (c + 1) * FD].T)  # (128, D)
        eb = np.empty((HPC, S, S), dtype=np.float32)
        for j in range(HPC):
            bT = bias[h0 + j].T.astype(np.float32)  # (k, q)
            eb[j] = np.where(maskT, np.exp(bT), np.float32(0))
        m = {
            "xT": xf, "wcat": wcat, "woB": wob,
            "expb": eb, "cosp": cosp, "sinp": sinp, "ident": identm,
            "onesd": np.ones((128, 64), dtype=np.float32),
        }
        if qk_bias:
            # rope applied to the constant bias vector, per position
            for name, bvec in (("bropeq", bq / np.sqrt(np.float32(DK))), ("bropek", bk)):
                bt = np.empty((128, R), dtype=np.float32)
                bb = bvec[fsl].astype(np.float32).reshape(HPC, DK // 2, 2)
                for j in range(HPC):
                    be = bb[j, :, 0][None, :]  # (1, 32)
                    bo_ = bb[j, :, 1][None, :]
                    rot_e = be * cs_full - bo_ * sn_full   # (R, 32)
                    rot_o = be * sn_full + bo_ * cs_full
                    blk = np.empty((R, DK), dtype=np.float32)
                    blk[:, 0::2] = rot_e
                    blk[:, 1::2] = rot_o
                    bt[j * DK:(j + 1) * DK, :] = blk.T
                m[name] = np.ascontiguousarray(bt)
        in_maps.append(m)
    return in_maps, qk_bias


def kernel(x, mask, bias, rope_freqs, Wq, bq, Wk, bk, Wv, bv, Wo, bo, **extra):
    x = np.asarray(x); mask = np.asarray(mask); bias = np.asarray(bias)
    rope_freqs = np.asarray(rope_freqs)
    Wq = np.asarray(Wq); bq = np.asarray(bq); Wk = np.asarray(Wk); bk = np.asarray(bk)
    Wv = np.asarray(Wv); bv = np.asarray(bv); Wo = np.asarray(Wo); bo = np.asarray(bo)

    causal = bool(np.array_equal(mask != 0, np.tril(np.ones((S, S), dtype=bool))))
    in_maps, qk_bias = _host_prep(x, mask, bias, rope_freqs, Wq, bq, Wk, bk, Wv, bv,
                                  Wo, bo, causal)
    nc = _get_kernel(causal, qk_bias)
    res = run_bass_kernel_spmd(nc, in_maps, list(range(NCORES)))
    acc = np.zeros((R, D), dtype=np.float32)
    for c in range(NCORES):
        acc += res.results[c]["out"]
    acc += bo.astype(np.float32)[None, :]
    if np.any(bv):
        acc += (bv.astype(np.float32) @ Wo.T.astype(np.float32))[None, :]
    return acc.reshape(B, S, D).astype(np.float32)


# revision 17
# speedup vs baseline: 1.2644x; 1.2644x over previous
"""Trainium2 Bass kernel for nn_MultiHeadAttention (B=2,S=2048,D=1024,H=16, RoPE+ALiBi+causal).

Strategy: head-parallel sharding across 8 NeuronCores (2 heads/core, both batches).
Host folds mask+bias into a per-head multiplicative exp(bias) tensor (bf16),
pre-transposes x, and sums the per-core partial output projections.
Matmul datapath is bf16 (fp32 PSUM accumulation); output partials are fp32.
"""
import sys, os

for _p in ("/root/.axon_site/_ro/trn_rl_repo", "/opt/trn_rl_repo"):
    if os.path.isdir(_p) and _p not in sys.path:
        sys.path.insert(0, _p)

import numpy as np
import ml_dtypes
import concourse.bass as bass
import concourse.mybir as mybir
import concourse.tile as tile
from concourse import bacc
from concourse.bass_utils import run_bass_kernel_spmd

F32 = mybir.dt.float32
BF16 = mybir.dt.bfloat16

B, S, D, H = 2, 2048, 1024, 16
DK = D // H            # 64
NCORES = 8
HPC = H // NCORES      # 2 heads per core
FD = HPC * DK          # 128 ctx features per core
R = B * S              # 4096 token rows
RT = R // 128          # 32 r-tiles
QB = 512               # q-block size
NQB = S // QB          # 4 q-blocks per batch


def _build(causal: bool, qk_bias: bool, stage: int = 2):
    """Build + compile the per-core SPMD kernel. Returns the compiled Bacc."""
    nc = bacc.Bacc()

    xT = nc.dram_tensor("xT", (D, R), BF16, kind="ExternalInput")
    wcat = nc.dram_tensor("wcat", (D, 3 * FD), BF16, kind="ExternalInput")
    woB = nc.dram_tensor("woB", (2 * DK, D), BF16, kind="ExternalInput")
    expb = nc.dram_tensor("expb", (HPC, S, S), BF16, kind="ExternalInput")
    cosp = nc.dram_tensor("cosp", (128, RT * (DK // 2)), BF16, kind="ExternalInput")
    sinp = nc.dram_tensor("sinp", (128, RT * (DK // 2)), BF16, kind="ExternalInput")
    ident = nc.dram_tensor("ident", (128, 128), BF16, kind="ExternalInput")
    onesd = nc.dram_tensor("onesd", (128, 64), F32, kind="ExternalInput")
    onesb = nc.dram_tensor("onesb", (128, 1), BF16, kind="ExternalInput")
    if qk_bias:
        bropeq = nc.dram_tensor("bropeq", (128, R), BF16, kind="ExternalInput")
        bropek = nc.dram_tensor("bropek", (128, R), BF16, kind="ExternalInput")
    out = nc.dram_tensor("out", (R, D), F32, kind="ExternalOutput")

    with tile.TileContext(nc) as tc:
        import contextlib
        ctx = contextlib.ExitStack()
        with ctx:
            consts = ctx.enter_context(tc.tile_pool(name="consts", bufs=1))
            persist = ctx.enter_context(tc.tile_pool(name="persist", bufs=1))

            # --- constants ---
            id_sb = consts.tile([128, 128], BF16)
            nc.sync.dma_start(out=id_sb, in_=ident[:, :])
            wc_sb = [consts.tile([128, 3 * FD], BF16, tag=f"wc{ct}", name=f"wc{ct}") for ct in range(8)]
            for ct in range(8):
                nc.sync.dma_start(out=wc_sb[ct], in_=wcat[ct * 128:(ct + 1) * 128, :])
            wo_sb = consts.tile([2 * DK, D], BF16, tag="wo", name="wo")
            nc.sync.dma_start(out=wo_sb, in_=woB[:, :])
            cos_sb = consts.tile([128, RT * 32], BF16)
            sin_sb = consts.tile([128, RT * 32], BF16)
            nc.sync.dma_start(out=cos_sb, in_=cosp[:, :])
            nc.sync.dma_start(out=sin_sb, in_=sinp[:, :])
            ones_sb = consts.tile([128, DK], F32)
            nc.sync.dma_start(out=ones_sb, in_=onesd[:, :])

            # --- persistent activation tensors ---
            QT = persist.tile([128, R], BF16, tag="QT")   # rows: h*64 + d, cols: b*2048+s
            KT = persist.tile([128, R], BF16, tag="KT")
            vaug = {}
            for rt in range(RT):
                for hh in range(HPC):
                    vaug[(rt, hh)] = persist.tile([128, DK + 1], BF16, tag=f"va{rt}_{hh}", name=f"va{rt}_{hh}")

            # =================== Phase 1: QKV projection + RoPE + transposes ===================
            with tc.tile_pool(name="p1x", bufs=3) as p1x, \
                 tc.tile_pool(name="p1n", bufs=1) as p1n, \
                 tc.tile_pool(name="p1s", bufs=4) as p1s, \
                 tc.tile_pool(name="p1ps", bufs=3, space="PSUM") as p1ps, \
                 tc.tile_pool(name="p1pt", bufs=3, space="PSUM") as p1pt:

                qknat = p1n.tile([128, RT * 256], BF16)  # col = rt*256 + {0:128 Q | 128:256 K}, d-major
                qkrot = p1n.tile([128, RT * 256], BF16)

                GRP = 8  # r-tiles per rope group
                for g in range(RT // GRP):
                    for rt in range(g * GRP, (g + 1) * GRP):
                        xts = [p1x.tile([128, 128], BF16, tag=f"x{ct}", name=f"xt{rt}_{ct}") for ct in range(8)]
                        for ct in range(8):
                            nc.sync.dma_start(
                                out=xts[ct],
                                in_=xT[ct * 128:(ct + 1) * 128, rt * 128:(rt + 1) * 128])
                        pp = p1ps.tile([128, 3 * FD], F32, tag="prj")
                        for ct in range(8):
                            nc.tensor.matmul(pp, xts[ct], wc_sb[ct],
                                             start=(ct == 0), stop=(ct == 7))
                        # drain Q,K -> qknat (bf16); V -> vaug tiles (bf16)
                        nc.vector.tensor_copy(qknat[:, rt * 256: rt * 256 + 256], pp[:, 0:256])
                        for hh in range(HPC):
                            va = vaug[(rt, hh)]
                            nc.scalar.copy(va[:, 0:DK], pp[:, 2 * FD + hh * DK: 2 * FD + (hh + 1) * DK])
                            nc.sync.dma_start(out=va[:, DK:DK + 1], in_=onesb[:, 0:1])

                    # rope on group g: Q and K separately, 6 ops each
                    # AP dims over qknat: [part][rt(8) step 256][head(2) step 64][pair(32) step 2]
                    def sl(t, qk, eo, g=g):
                        a = t[:, :]
                        return bass.AP(
                            tensor=a.tensor,
                            offset=a.offset + (g * GRP * 256 + qk * 128 + eo),
                            ap=[a.ap[0], [256, GRP], [64, 2], [2, 32]],
                        )
                    def slc(t, g=g):  # cos/sin: [part][rt(8) step 32][head 0x2][pair step 1 x32]
                        a = t[:, :]
                        return bass.AP(
                            tensor=a.tensor,
                            offset=a.offset + g * GRP * 32,
                            ap=[a.ap[0], [32, GRP], [0, 2], [1, 32]],
                        )
                    for qk in range(2):
                        s1 = p1s.tile([128, GRP * 64], BF16, tag="s1")
                        s2 = p1s.tile([128, GRP * 64], BF16, tag="s2")
                        s3 = p1s.tile([128, GRP * 64], BF16, tag="s3")
                        s4 = p1s.tile([128, GRP * 64], BF16, tag="s4")
                        nc.vector.tensor_mul(s1, sl(qknat, qk, 0), slc(cos_sb))
                        nc.vector.tensor_mul(s2, sl(qknat, qk, 1), slc(sin_sb))
                        nc.vector.tensor_sub(sl(qkrot, qk, 0), s1, s2)
                        nc.vector.tensor_mul(s3, sl(qknat, qk, 0), slc(sin_sb))
                        nc.vector.tensor_mul(s4, sl(qknat, qk, 1), slc(cos_sb))
                        nc.vector.tensor_add(sl(qkrot, qk, 1), s3, s4)

                    # transposes for group g: (128r x 128d) -> (128d x 128r)
                    for rt in range(g * GRP, (g + 1) * GRP):
                        for qk, dst in ((0, QT), (1, KT)):
                            pt = p1pt.tile([128, 128], BF16, tag="pt")
                            nc.tensor.transpose(pt, qkrot[:, rt * 256 + qk * 128: rt * 256 + qk * 128 + 128], id_sb)
                            nc.scalar.copy(dst[:, rt * 128:(rt + 1) * 128], pt)

                if qk_bias:
                    brq = p1n.tile([128, R], BF16, tag="brq")
                    brk = p1n.tile([128, R], BF16, tag="brk")
                    nc.sync.dma_start(out=brq, in_=bropeq[:, :])
                    nc.sync.dma_start(out=brk, in_=bropek[:, :])
                    nc.vector.tensor_add(QT, QT, brq)
                    nc.vector.tensor_add(KT, KT, brk)

            if stage == 1:
                for i in range(4):
                    nc.gpsimd.dma_start(out=out[i * 128:(i + 1) * 128, :],
                                        in_=QT[:, i * 1024:(i + 1) * 1024])
                    nc.gpsimd.dma_start(out=out[512 + i * 128: 512 + (i + 1) * 128, :],
                                        in_=KT[:, i * 1024:(i + 1) * 1024])
                nc.gpsimd.dma_start(out=out[1024:1152, 0:65], in_=vaug[(0, 0)])
                nc.gpsimd.dma_start(out=out[1152:1280, 0:65], in_=vaug[(31, 1)])
            # =================== Phase 2: attention + output projection ===================
            if stage >= 2:
              with tc.tile_pool(name="p2eb", bufs=3) as p2eb, \
                 tc.tile_pool(name="p2e", bufs=3) as p2e, \
                 tc.tile_pool(name="p2a", bufs=3) as p2a, \
                 tc.tile_pool(name="p2c", bufs=2) as p2c, \
                 tc.tile_pool(name="p2o", bufs=3) as p2o, \
                 tc.tile_pool(name="psc", bufs=3, space="PSUM") as psc, \
                 tc.tile_pool(name="psx", bufs=1, space="PSUM") as psx, \
                 tc.tile_pool(name="pm", bufs=1, space="PSUM") as pm:

                for qb in range(NQB):
                    nkt = (qb + 1) * (QB // 128) if causal else S // 128
                    ctx_ps = {}
                    for hh in range(HPC):
                        for b in range(B):
                            ctx_ps[(hh, b)] = psx.tile([DK + 1, QB], F32, tag=f"ctx{hh}{b}", name=f"ctx{qb}_{hh}{b}")
                    for kt in range(nkt):
                        q_off = max(0, kt * 128 - qb * QB) if causal else 0
                        q_len = QB - q_off
                        for hh in range(HPC):
                            ebt = p2eb.tile([128, QB], BF16, tag=f"eb{hh}")
                            nc.sync.dma_start(
                                out=ebt[:, 0:q_len],
                                in_=expb[hh, kt * 128:(kt + 1) * 128,
                                         qb * QB + q_off: (qb + 1) * QB])
                            for b in range(B):
                                scp = psc.tile([128, QB], F32, tag="sc")
                                nc.tensor.matmul(
                                    scp[:, 0:q_len],
                                    KT[hh * DK:(hh + 1) * DK, b * S + kt * 128: b * S + (kt + 1) * 128],
                                    QT[hh * DK:(hh + 1) * DK, b * S + qb * QB + q_off: b * S + (qb + 1) * QB],
                                    start=True, stop=True)
                                ex = p2e.tile([128, QB], BF16, tag="ex")
                                nc.scalar.activation(ex[:, 0:q_len], scp[:, 0:q_len],
                                                     mybir.ActivationFunctionType.Exp)
                                at = p2a.tile([128, QB], BF16, tag="at")
                                nc.vector.tensor_mul(at[:, 0:q_len], ex[:, 0:q_len], ebt[:, 0:q_len])
                                nc.tensor.matmul(
                                    ctx_ps[(hh, b)][:, q_off:QB],
                                    vaug[(b * (S // 128) + kt, hh)],
                                    at[:, 0:q_len],
                                    start=(kt == 0), stop=(kt == nkt - 1))
                    if stage == 3:
                        for hh in range(HPC):
                            for b in range(B):
                                dt_sb = p2o.tile([DK + 1, QB], F32, tag="dmp", name=f"dmp{qb}{hh}{b}")
                                nc.vector.tensor_copy(dt_sb, ctx_ps[(hh, b)])
                                nc.sync.dma_start(
                                    out=out[(qb * 4 + hh * 2 + b) * 128: (qb * 4 + hh * 2 + b) * 128 + DK + 1, 0:QB],
                                    in_=dt_sb)
                        continue
                    # normalize + output projection
                    csc = {}
                    for b in range(B):
                        cboth = p2c.tile([2 * DK, QB], BF16, tag=f"cb{b}", name=f"cb{qb}{b}")
                        for hh in range(HPC):
                            cp = ctx_ps[(hh, b)]
                            rc = p2c.tile([DK + 1, QB], F32, tag=f"rc{hh}{b}")
                            lg = p2c.tile([DK + 1, QB], F32, tag=f"lg{hh}{b}")
                            nc.scalar.activation(lg[DK:DK + 1, :], cp[DK:DK + 1, :],
                                                 mybir.ActivationFunctionType.Ln)
                            nc.scalar.activation(rc[DK:DK + 1, :], lg[DK:DK + 1, :],
                                                 mybir.ActivationFunctionType.Exp, scale=-1.0)
                            rbp = pm.tile([DK, QB], F32, tag="rb")
                            nc.tensor.matmul(rbp, ones_sb[DK:DK + 1, :], rc[DK:DK + 1, :],
                                             start=True, stop=True)
                            rb = p2c.tile([DK, QB], F32, tag=f"rb{hh}{b}")
                            nc.scalar.copy(rb, rbp)
                            if hh == 0:
                                nc.vector.tensor_mul(cboth[0:DK, :], cp[0:DK, :], rb)
                            else:
                                cs1 = p2c.tile([DK, QB], BF16, tag=f"cs{hh}{b}")
                                nc.vector.tensor_mul(cs1, cp[0:DK, :], rb)
                                nc.sync.dma_start(out=cboth[DK:2 * DK, :], in_=cs1)
                        csc[b] = cboth
                    for b in range(B):
                        for rs in range(QB // 128):
                            ot = p2o.tile([128, D], F32, tag="ot")
                            for eh in range(2):
                                op = psc.tile([128, 512], F32, tag="sc")
                                nc.tensor.matmul(op, csc[b][:, rs * 128:(rs + 1) * 128],
                                                 wo_sb[:, eh * 512:(eh + 1) * 512],
                                                 start=True, stop=True)
                                if eh == 0:
                                    nc.vector.tensor_copy(ot[:, 0:512], op)
                                else:
                                    nc.scalar.copy(ot[:, 512:1024], op)
                            nc.sync.dma_start(
                                out=out[b * S + qb * QB + rs * 128: b * S + qb * QB + (rs + 1) * 128, :],
                                in_=ot)
    nc.compile()
    return nc


_CACHE = {}


def _get_kernel(causal: bool, qk_bias: bool):
    key = (causal, qk_bias)
    if key not in _CACHE:
        _CACHE[key] = _build(causal, qk_bias)
    return _CACHE[key]


def _host_prep(x, mask, bias, rope_freqs, Wq, bq, Wk, bk, Wv, bv, Wo, bo, causal):
    """Build the 8 per-core input maps."""
    bf = ml_dtypes.bfloat16
    xf = np.ascontiguousarray(x.reshape(R, D).T.astype(bf))  # (D, R)
    cosf = np.cos(rope_freqs.astype(np.float32))  # (S, 32)
    sinf = np.sin(rope_freqs.astype(np.float32))
    rr = np.arange(R)
    cs_full = cosf[rr % S]  # (R, 32)
    sn_full = sinf[rr % S]
    cosp = np.ascontiguousarray(
        cs_full.reshape(RT, 128, 32).transpose(1, 0, 2).reshape(128, RT * 32).astype(bf))
    sinp = np.ascontiguousarray(
        sn_full.reshape(RT, 128, 32).transpose(1, 0, 2).reshape(128, RT * 32).astype(bf))
    identm = np.eye(128, dtype=np.float32).astype(bf)

    qk_bias = bool(np.any(bq) or np.any(bk))
    maskT = (mask != 0).T  # (k, q)

    in_maps = []
    for c in range(NCORES):
        h0 = c * HPC
        fsl = slice(c * FD, (c + 1) * FD)
        wq = Wq[fsl, :].astype(np.float32) / np.sqrt(np.float32(DK))
        wk = Wk[fsl, :].astype(np.float32)
        wv = Wv[fsl, :].astype(np.float32)
        wcat = np.ascontiguousarray(np.concatenate([wq, wk, wv], axis=0).T.astype(bf))  # (D, 384)
        wob = np.ascontiguousarray(Wo[:, c * FD: (c + 1) * FD].T.astype(bf))  # (128, D)
        eb = np.empty((HPC, S, S), dtype=bf)
        for j in range(HPC):
            bT = bias[h0 + j].T.astype(np.float32)  # (k, q)
            eb[j] = np.where(maskT, np.exp(np.minimum(bT, np.float32(80.0))),
                             np.float32(0)).astype(bf)
        m = {
            "xT": xf, "wcat": wcat, "woB": wob,
            "expb": eb, "cosp": cosp, "sinp": sinp, "ident": identm,
            "onesd": np.ones((128, 64), dtype=np.float32),
            "onesb": np.ones((128, 1), dtype=bf),
        }
        if qk_bias:
            for name, bvec in (("bropeq", bq / np.sqrt(np.float32(DK))), ("bropek", bk)):
                bt = np.empty((128, R), dtype=np.float32)
                bb = bvec[fsl].astype(np.float32).reshape(HPC, DK // 2, 2)
                for j in range(HPC):
                    be = bb[j, :, 0][None, :]  # (1, 32)
                    bo_ = bb[j, :, 1][None, :]
                    rot_e = be * cs_full - bo_ * sn_full   # (R, 32)
                    rot_o = be * sn_full + bo_ * cs_full
                    blk = np.empty((R, DK), dtype=np.float32)
                    blk[:, 0::2] = rot_e
                    blk[:, 1::2] = rot_o
                    bt[j * DK:(j + 1) * DK, :] = blk.T
                m[name] = np.ascontiguousarray(bt.astype(bf))
        in_maps.append(m)
    return in_maps, qk_bias


def kernel(x, mask, bias, rope_freqs, Wq, bq, Wk, bk, Wv, bv, Wo, bo, **extra):
    x = np.asarray(x); mask = np.asarray(mask); bias = np.asarray(bias)
    rope_freqs = np.asarray(rope_freqs)
    Wq = np.asarray(Wq); bq = np.asarray(bq); Wk = np.asarray(Wk); bk = np.asarray(bk)
    Wv = np.asarray(Wv); bv = np.asarray(bv); Wo = np.asarray(Wo); bo = np.asarray(bo)

    causal = bool(np.array_equal(mask != 0, np.tril(np.ones((S, S), dtype=bool))))
    in_maps, qk_bias = _host_prep(x, mask, bias, rope_freqs, Wq, bq, Wk, bk, Wv, bv,
                                  Wo, bo, causal)
    nc = _get_kernel(causal, qk_bias)
    res = run_bass_kernel_spmd(nc, in_maps, list(range(NCORES)))
    acc = np.zeros((R, D), dtype=np.float32)
    for c in range(NCORES):
        acc += res.results[c]["out"]
    acc += bo.astype(np.float32)[None, :]
    if np.any(bv):
        acc += (bv.astype(np.float32) @ Wo.T.astype(np.float32))[None, :]
    return acc.reshape(B, S, D).astype(np.float32)
